# revision 20
# baseline (speedup 1.0000x reference)
"""Feedforward SNN (Linear -> LIF) x2 kernel for Trainium2, 8-core data parallel.

Per-core plan (B sharded 8 ways, BL=32 samples/core):
  - Host pre-transposes operands once (cheap numpy): xT[d, (t,b)] per core,
    W1T[d, h1] fp32, and W2 split Dekker-style into two bf16 terms stored
    interleaved+transposed W2T_hl[h1, 2, h2]. The device runs ONLY matmuls
    and LIF scans -- no on-chip transposes except the tiny output unscramble.
  - Layer-1 currents for ALL timesteps: Cur1[h1, (t,b)] = W1 @ x^T (fp32;
    x does not depend on recurrent state).
  - LIF-1 scan over t on [128, HC1*32] tiles (partition = h1 % 128, free =
    (h1chunk, b)); fused scalar_tensor_tensor DVE ops, 3/step.
  - Spikes are {0,1} == exact in bf16; layer-2 currents are 2x bf16 matmuls
    (W2h + W2l) accumulated in fp32 PSUM -- 2x faster than fp32 matmul,
    error at fp32-reorder noise level (validated vs reference envelope).
  - LIF-2 scan likewise (2 DVE ops/step; spikes materialized only at t=63).
  - Software-pipelined: mm1(nb+1) is emitted before mm2(nb) so the PE fills
    the scan1(nb) latency; PE phases are chained with order-only deps.
"""

import os
import sys

import numpy as np

for _p in ("/opt/trn_rl_repo", "/root/.axon_site/_ro/trn_rl_repo"):
    if os.path.isdir(_p) and _p not in sys.path:
        sys.path.insert(0, _p)

import ml_dtypes  # noqa: E402

import concourse.bass as bass  # noqa: E402
import concourse.mybir as mybir  # noqa: E402
import concourse.tile as tile  # noqa: E402
from concourse import bacc  # noqa: E402
from concourse.bass_utils import run_bass_kernel_spmd  # noqa: E402
from concourse.masks import make_identity  # noqa: E402
from concourse.tile_rust import add_dep_helper  # noqa: E402

F32 = mybir.dt.float32
F32R = mybir.dt.float32r
BF16 = mybir.dt.bfloat16
ALU = mybir.AluOpType
AF = mybir.ActivationFunctionType

BETA = 0.9
THR = 1.0

B_FULL, T_FULL, D_FULL, H1_FULL, H2_FULL = 256, 64, 1024, 2048, 2048
N_CORES = 8
BL = B_FULL // N_CORES  # 32


def build_snn(T=T_FULL, D=D_FULL, H1=H1_FULL, H2=H2_FULL, T_NB=16):
    """Build the single-core Bass program (identical across the 8 cores)."""
    P = 128
    KC1 = D // P
    HC1 = H1 // P
    HC2 = H2 // P
    NNB = T // T_NB
    SUB = min(4, T_NB)
    NSUB = T_NB // SUB
    MCQ = min(4, HC2)
    HCQ = min(4, HC1)
    NB32 = T_NB * 32          # matmul free dim per t-block

    assert T % T_NB == 0 and T_NB % SUB == 0
    assert HC2 % MCQ == 0 and HC1 % HCQ == 0

    nc = bacc.Bacc("TRN2", target_bir_lowering=False, debug=False)

    xt_d = nc.dram_tensor("xT", [D, T * BL], F32, kind="ExternalInput")
    w1t_d = nc.dram_tensor("W1T", [D, H1], F32, kind="ExternalInput")
    b1_d = nc.dram_tensor("b1", [H1], F32, kind="ExternalInput")
    w2t_d = nc.dram_tensor("W2Thl", [H1, 2, H2], BF16, kind="ExternalInput")
    b2_d = nc.dram_tensor("b2", [H2], F32, kind="ExternalInput")

    spk2_d = nc.dram_tensor("spk2", [BL, H2], F32, kind="ExternalOutput")
    mem1_d = nc.dram_tensor("mem1", [BL, H1], F32, kind="ExternalOutput")
    mem2_d = nc.dram_tensor("mem2", [BL, H2], F32, kind="ExternalOutput")

    with tile.TileContext(nc) as tc:
        from contextlib import ExitStack
        ctx = ExitStack()
        with ctx:
            const = ctx.enter_context(tc.tile_pool(name="const", bufs=1))
            xtp = ctx.enter_context(tc.tile_pool(name="xtp", bufs=2))
            w1tp = ctx.enter_context(tc.tile_pool(name="w1tp", bufs=4))
            w2tp = ctx.enter_context(tc.tile_pool(name="w2tp", bufs=4))
            curp = ctx.enter_context(tc.tile_pool(name="curp", bufs=5))
            spk1p = ctx.enter_context(tc.tile_pool(name="spk1p", bufs=1))
            statep = ctx.enter_context(tc.tile_pool(name="statep", bufs=2))
            negzp = ctx.enter_context(tc.tile_pool(name="negzp", bufs=1))
            outp = ctx.enter_context(tc.tile_pool(name="outp", bufs=4))
            tpsum = ctx.enter_context(
                tc.tile_pool(name="tpsum", bufs=2, space="PSUM"))
            mpsum = ctx.enter_context(
                tc.tile_pool(name="mpsum", bufs=6, space="PSUM"))

            ident = const.tile([P, P], F32, name="ident")
            make_identity(nc, ident)

            # PE phase chaining (order-only deps): keeps fp32-mm, bf16-mm
            # and transpose phases from interleaving in the PE stream.
            pe_phases = []

            class _Ph:
                def __init__(self):
                    self.insts = []

                def add(self, bi):
                    self.insts.append(bi.ins)

            b1s = const.tile([P, HC1], F32, name="b1s")
            nc.gpsimd.dma_start(
                b1s[:], b1_d.ap().rearrange("(c p) -> p c", p=P))
            b2s = const.tile([P, HC2], F32, name="b2s")
            nc.gpsimd.dma_start(
                b2s[:], b2_d.ap().rearrange("(c p) -> p c", p=P))

            # ---------------- initial LIF state ----------------------------
            mem1_cur = statep.tile([P, HC1, 32], F32, tag="mem1",
                                   name="mem1_0")
            nc.vector.memset(mem1_cur[:], 0.0)
            mem2_cur = statep.tile([P, HC2, 32], F32, tag="mem2",
                                   name="mem2_0")
            nc.vector.memset(mem2_cur[:], 0.0)
            spk2_fin = const.tile([P, HC2, 32], F32, name="spk2_fin")

            # ---------------- outputs helper --------------------------------
            def emit_out(state, nch, out_d):
                ph = _Ph()
                pe_phases.append(ph)
                for hc in range(nch):
                    ps = tpsum.tile([32, P], F32, tag="tp", name="ops")
                    ph.add(nc.tensor.transpose(ps[:], state[:, hc, :],
                                               ident[:]))
                    sb = outp.tile([32, P], F32, tag="osb", name="osb")
                    nc.scalar.activation(sb[:], ps[:], AF.Copy)
                    nc.sync.dma_start(
                        out_d.ap()[:, hc * P:(hc + 1) * P], sb[:])

            # ---------------- per-block emitters ----------------------------
            def x_and_mm1(nb):
                """xT load + matmul1 for block nb -> cur1_subs"""
                ph = _Ph()
                pe_phases.append(ph)
                t0 = nb * T_NB
                xt = xtp.tile([P, KC1, NB32], F32, tag="xt", name="xt")
                for kc in range(KC1):
                    nc.gpsimd.dma_start(
                        xt[:, kc, :],
                        xt_d.ap()[kc * P:(kc + 1) * P,
                                  t0 * 32:(t0 + T_NB) * 32])

                cur1_subs = [curp.tile([P, SUB, HC1, 32], F32, tag="cur1",
                                       bufs=7, name="cur1")
                             for _ in range(NSUB)]
                for hq in range(HC1 // HCQ):
                    pss = [mpsum.tile([P, NB32], F32, tag="mm", name="mm1ps")
                           for _ in range(HCQ)]
                    for kc in range(KC1):
                        w1tt = w1tp.tile([P, HCQ * P], F32, tag="w1t",
                                         name="w1tt")
                        dq = nc.sync if kc % 2 == 0 else nc.scalar
                        dq.dma_start(
                            w1tt[:],
                            w1t_d.ap()[kc * P:(kc + 1) * P,
                                       hq * HCQ * P:(hq + 1) * HCQ * P])
                        rhs = xt[:, kc, :]
                        for i in range(HCQ):
                            ph.add(nc.tensor.matmul(
                                pss[i][:], w1tt[:, i * P:(i + 1) * P], rhs,
                                start=(kc == 0), stop=(kc == KC1 - 1)))
                    for i in range(HCQ):
                        hc = hq * HCQ + i
                        psv = pss[i].rearrange("p (t b) -> p t b", b=32)
                        for s in range(NSUB):
                            nc.scalar.activation(
                                cur1_subs[s][:, :, hc, :],
                                psv[:, s * SUB:(s + 1) * SUB, :],
                                AF.Identity, bias=b1s[:, hc:hc + 1])
                return cur1_subs

            # ---------------- main t-block pipeline -------------------------
            # software pipelining: mm1(nb+1) is emitted BEFORE mm2(nb) so the
            # PE stream (priority = program order) fills the scan1(nb)
            # latency with mm1(nb+1) instead of stalling on spk1.
            cur1_next = x_and_mm1(0)
            for nb in range(NNB):
                t0 = nb * T_NB
                cur1_subs = cur1_next
                if nb + 1 < NNB:
                    cur1_next = x_and_mm1(nb + 1)

                # -- scan1 (T_NB steps); spikes (bf16) into spk1[(kc,t,b)] ---
                spk1 = spk1p.tile([P, HC1, NB32], BF16, tag="spk1",
                                  name="spk1")
                for tr in range(T_NB):
                    cur_t = cur1_subs[tr // SUB][:, tr % SUB]  # [P, HC1, 32]
                    negz = negzp.tile([P, HC1, 32], F32, tag="negz1",
                                      name="negz1")
                    nc.vector.scalar_tensor_tensor(
                        negz[:], mem1_cur[:], THR, cur_t,
                        ALU.is_gt, ALU.subtract)
                    mem1_new = statep.tile([P, HC1, 32], F32, tag="mem1",
                                           name="mem1")
                    nc.vector.scalar_tensor_tensor(
                        mem1_new[:], mem1_cur[:], BETA, negz[:],
                        ALU.mult, ALU.subtract)
                    mem1_cur = mem1_new
                    # spike of step t thresholds the POST-update membrane
                    nc.vector.tensor_scalar(
                        spk1[:, :, tr * 32:(tr + 1) * 32], mem1_cur[:],
                        THR, None, ALU.is_gt)

                if nb == NNB - 1:
                    emit_out(mem1_cur, HC1, mem1_d)

                # -- matmul2 (2x bf16): cur2[(t,mc,b)] = W2 @ spk1^T + b2 ----
                ph = _Ph()
                pe_phases.append(ph)
                cur2_subs = [curp.tile([P, SUB, HC2, 32], F32, tag="cur2",
                                       name="cur2") for _ in range(NSUB)]
                for mq in range(HC2 // MCQ):
                    pss = [mpsum.tile([P, NB32], F32, tag="mm", name="mm2ps")
                           for _ in range(MCQ)]
                    for kc in range(HC1):
                        wt = w2tp.tile([P, 2, MCQ * P], BF16, tag="w2t",
                                       name="w2t")
                        dq = nc.sync if kc % 2 == 0 else nc.scalar
                        dq.dma_start(
                            wt[:],
                            w2t_d.ap()[kc * P:(kc + 1) * P, :,
                                       mq * MCQ * P:(mq + 1) * MCQ * P])
                        rhs = spk1[:, kc, :]
                        for i in range(MCQ):
                            ph.add(nc.tensor.matmul(
                                pss[i][:], wt[:, 0, i * P:(i + 1) * P], rhs,
                                start=(kc == 0), stop=False))
                            ph.add(nc.tensor.matmul(
                                pss[i][:], wt[:, 1, i * P:(i + 1) * P], rhs,
                                start=False, stop=(kc == HC1 - 1)))
                    for i in range(MCQ):
                        mc = mq * MCQ + i
                        psv = pss[i].rearrange("p (t b) -> p t b", b=32)
                        for s in range(NSUB):
                            nc.scalar.activation(
                                cur2_subs[s][:, :, mc, :],
                                psv[:, s * SUB:(s + 1) * SUB, :],
                                AF.Identity, bias=b2s[:, mc:mc + 1])

                # -- scan2 (T_NB steps) --------------------------------------
                for tr in range(T_NB):
                    t = t0 + tr
                    cur_t = cur2_subs[tr // SUB][:, tr % SUB]
                    negz = negzp.tile([P, HC2, 32], F32, tag="negz2",
                                      name="negz2")
                    nc.vector.scalar_tensor_tensor(
                        negz[:], mem2_cur[:], THR, cur_t,
                        ALU.is_gt, ALU.subtract)
                    mem2_new = statep.tile([P, HC2, 32], F32, tag="mem2",
                                           name="mem2")
                    nc.vector.scalar_tensor_tensor(
                        mem2_new[:], mem2_cur[:], BETA, negz[:],
                        ALU.mult, ALU.subtract)
                    mem2_cur = mem2_new
                    if t == T - 1:
                        nc.vector.tensor_scalar(
                            spk2_fin[:], mem2_cur[:], THR, None, ALU.is_gt)

            # ---------------- remaining outputs -----------------------------
            emit_out(mem2_cur, HC2, mem2_d)
            emit_out(spk2_fin, HC2, spk2_d)

            # chain consecutive PE phases: every inst of phase b ordered
            # after the last inst of phase a (order-only deps)
            for a, b in zip(pe_phases, pe_phases[1:]):
                if a.insts and b.insts:
                    for bi in b.insts:
                        add_dep_helper(bi, a.insts[-1], sync=False,
                                       reason="PE phase ordering")

    nc.compile()
    return nc


_NC_CACHE = {}


def _get_nc():
    if "full" not in _NC_CACHE:
        _NC_CACHE["full"] = build_snn()
    return _NC_CACHE["full"]


def prep_inputs(x, W1, b1, W2, b2):
    """Host-side prep: shard x over cores (transposed to [d, (t,b)]),
    transpose W1, and split W2 into two transposed+interleaved bf16 terms."""
    x = np.asarray(x, np.float32)
    W1 = np.asarray(W1, np.float32)
    b1 = np.ascontiguousarray(np.asarray(b1, np.float32))
    W2 = np.asarray(W2, np.float32)
    b2 = np.ascontiguousarray(np.asarray(b2, np.float32))
    B, T, D = x.shape

    W1T = np.ascontiguousarray(W1.T)                    # [D, H1]
    W2T = np.ascontiguousarray(W2.T)                    # [H1, H2]
    W2Th = W2T.astype(ml_dtypes.bfloat16)
    W2Tl = (W2T - W2Th.astype(np.float32)).astype(ml_dtypes.bfloat16)
    W2Thl = np.ascontiguousarray(
        np.stack([W2Th, W2Tl], axis=1))                 # [H1, 2, H2]

    bl = B // N_CORES
    in_maps = []
    for c in range(N_CORES):
        xc = x[c * bl:(c + 1) * bl]                     # [bl, T, D]
        xT = np.ascontiguousarray(
            xc.transpose(2, 1, 0).reshape(D, T * bl))   # [d, (t,b)] t-major
        in_maps.append({
            "xT": xT, "W1T": W1T, "b1": b1, "W2Thl": W2Thl, "b2": b2,
        })
    return in_maps


def kernel(x, W1, b1, W2, b2):
    """Full-input entry point: shards B across 8 NeuronCores, returns full
    (spk2, mem1, mem2) exactly like reference()."""
    nc = _get_nc()
    in_maps = prep_inputs(x, W1, b1, W2, b2)
    res = run_bass_kernel_spmd(nc, in_maps, core_ids=list(range(N_CORES)))
    spk2 = np.concatenate([res.results[c]["spk2"] for c in range(N_CORES)], 0)
    mem1 = np.concatenate([res.results[c]["mem1"] for c in range(N_CORES)], 0)
    mem2 = np.concatenate([res.results[c]["mem2"] for c in range(N_CORES)], 0)
    return spk2, mem1, mem2
